# revision 19
# baseline (speedup 1.0000x reference)
"""BiLSTM tagger on 8 TRN2 NeuronCores.

Strategy (hardcoded for B=64,T=512,V=30000,E=128,H=256,TAGS=50):
  - Data-parallel: batch sharded 8 ways (8 sequences/core); weights replicated.
  - Per core: embedding gather (indirect DMA) -> PE transpose -> x^T in SBUF;
    input projections xg = W_ih_aug @ [x; 1-m; 1] precomputed for all t as big
    matmuls, stored bf16 in DRAM scratch; recurrences (l1 fwd+bwd interleaved,
    then l2 fwd+bwd) as For_i loops with 32 steps/body; classifier matmul.
  - Masking: a (1-m) aug feature adds +/-60 to the i/f gate pre-activations at
    masked steps, freezing c exactly. Backward h is then exactly 0 at masked
    steps. Forward l2 h is repaired POST-HOC with a tensor_tensor_scan along T
    (o2[t] = (1-m)*o2[t-1] + m*h[t]) instead of in-loop blending.
  - Gate trick: tanh(g) = 2*sigmoid(2g)-1, with the 2x pre-scaled into the
    g rows of the weights, so ONE sigmoid covers all 8 gate chunks straight
    from PSUM; a cheap tensor_scalar (2x-1) fixes up g.
  - xg is accumulated into PSUM by an identity matmul (start=True) before the
    Whh matmuls, so no separate PSUM+xg vector add is needed.
  - Gate layout: gates on partitions (8 chunks of 128 = [i0 i1 f0 f1 o0 o1 g0 g1]),
    batch on free dim; Whh stationary [128h x 128gate] bf16 tiles (FWL),
    h moving [128, 8].
"""
import sys

sys.path.insert(0, "/opt/trn_rl_repo")
import contextlib

import numpy as np
import ml_dtypes

import concourse.bass as bass
import concourse.bacc as bacc
import concourse.mybir as mybir
import concourse.tile as tile
from concourse.bass import ds
from concourse.bass_utils import run_bass_kernel_spmd
from concourse.masks import make_identity

B, T, V, E, H, TAGS = 64, 512, 30000, 128, 256, 50
NCORES = 8
Bc = B // NCORES          # 8 sequences per core
TB = T * Bc               # 4096 tokens per core
SPB = 64                  # steps per body
NBODY = T // SPB          # 8

f32 = mybir.dt.float32
bf16 = mybir.dt.bfloat16
i32 = mybir.dt.int32

UNITS = ("1f", "1b", "2f", "2b")
KCNT = {"1f": 1, "1b": 1, "2f": 4, "2b": 4}       # 128-row K chunks of x features
REV = {"1f": False, "1b": True, "2f": False, "2b": True}

# gate chunk order i0 i1 f0 f1 o0 o1 g0 g1 (torch row order is i f g o)
PERM = np.concatenate([np.arange(0, 256), np.arange(256, 512),
                       np.arange(768, 1024), np.arange(512, 768)])

_CACHE = {}


def _prep_unit_weights(Wih, Whh, bih, bhh):
    """Host-side weight marshalling for one LSTM direction."""
    din = Wih.shape[1]
    Wp = np.asarray(Wih)[PERM].astype(np.float64)   # [1024, din]
    Up = np.asarray(Whh)[PERM].astype(np.float64)   # [1024, 256]
    bp = (np.asarray(bih, np.float64) + np.asarray(bhh, np.float64))[PERM]
    k_cnt = din // 128
    # x-part lhsT: [din, 1024] -> k-chunk-major cols [128, k_cnt*8*128]
    WT = Wp.T.astype(np.float32)                    # [din, 1024]
    wx = np.concatenate([WT[k * 128:(k + 1) * 128, :] for k in range(k_cnt)],
                        axis=1).astype(ml_dtypes.bfloat16)  # [128, k_cnt*1024]
    # aug lhsT rows: feature0 = (1-m), feature1 = 1
    wa = np.zeros((2, 1024), np.float32)
    wa[0, 0:256] = -60.0   # i rows: -60*(1-m)
    wa[0, 256:512] = 60.0  # f rows: +60*(1-m)
    wa[1, :] = bp.astype(np.float32)
    wa = wa.astype(ml_dtypes.bfloat16)
    # Whh lhsT: [256, 1024] -> [128, 2*1024]
    UT = Up.T.astype(np.float32)
    wh = np.concatenate([UT[0:128, :], UT[128:256, :]], axis=1).astype(ml_dtypes.bfloat16)
    return wx, wa, wh


def _build_program():
    nc = bacc.Bacc("TRN2", target_bir_lowering=False, debug=False, num_devices=NCORES)
    emb_d = nc.dram_tensor("emb", [V, E], f32, kind="ExternalInput")
    words_d = nc.dram_tensor("words", [TB, 1], i32, kind="ExternalInput")
    aug_d = nc.dram_tensor("aug", [2, TB], bf16, kind="ExternalInput")
    hostm_d = nc.dram_tensor("hostm", [2, Bc, T], bf16, kind="ExternalInput")  # [m; 1-m]
    wxd, wad, whd, xgd = {}, {}, {}, {}
    for u in UNITS:
        wxd[u] = nc.dram_tensor(f"w{u}x", [128, KCNT[u] * 1024], bf16, kind="ExternalInput")
        wad[u] = nc.dram_tensor(f"w{u}a", [2, 1024], bf16, kind="ExternalInput")
        whd[u] = nc.dram_tensor(f"w{u}h", [128, 2048], bf16, kind="ExternalInput")
        xgd[u] = nc.dram_tensor(f"xg{u}", [128, 8, TB], bf16)
    clsx_d = nc.dram_tensor("clsx", [128, 4 * TAGS], bf16, kind="ExternalInput")
    clsb_d = nc.dram_tensor("clsb", [TAGS, 1], f32, kind="ExternalInput")
    logits_d = nc.dram_tensor("logits", [TAGS, TB], f32, kind="ExternalOutput")

    ctx = contextlib.ExitStack()
    with tile.TileContext(nc) as tc, ctx:
        pp = ctx.enter_context(tc.tile_pool(name="persist", bufs=1))
        xT = pp.tile([128, TB], bf16, tag="xT")
        aug_sb = pp.tile([2, TB], bf16, tag="aug")
        ident = pp.tile([128, 128], f32, tag="ident")
        identb = pp.tile([128, 128], bf16, tag="identb")
        wx_sb = {u: pp.tile([128, KCNT[u] * 1024], bf16, tag=f"wx{u}", name=f"wx{u}") for u in UNITS}
        wa_sb = {u: pp.tile([2, 1024], bf16, tag=f"wa{u}", name=f"wa{u}") for u in UNITS}
        wh_sb = {u: pp.tile([128, 2048], bf16, tag=f"wh{u}", name=f"wh{u}") for u in UNITS}
        cls_sb = pp.tile([128, 4 * TAGS], bf16, tag="clsx")
        clsb_sb = pp.tile([TAGS, 1], f32, tag="clsb")
        # h history with boundary slots: slot t+1 = h(t); slot 0 = h(-1) = 0,
        # slot T+1 = h(T) = 0 (for the reverse direction's first step)
        hs = {u: pp.tile([128, T + 2, 2, Bc], bf16, tag=f"hs{u}", name=f"hs{u}") for u in UNITS}
        o2rep = pp.tile([128, T, 2, Bc], bf16, tag="o2rep")
        ccar = {u: pp.tile([128, 2, Bc], f32, tag=f"cc{u}", name=f"cc{u}") for u in UNITS}
        mrep = pp.tile([128, 2, Bc, T], bf16, tag="mrep")  # [m; 1-m] bcast over partitions

        # ---- load weights / constants
        make_identity(nc, ident[:])
        nc.vector.tensor_copy(identb[:], ident[:])
        for u in UNITS:
            nc.sync.dma_start(wx_sb[u][:], wxd[u][:])
            nc.sync.dma_start(wa_sb[u][:], wad[u][:])
            nc.sync.dma_start(wh_sb[u][:], whd[u][:])
        nc.sync.dma_start(cls_sb[:], clsx_d[:])
        nc.sync.dma_start(clsb_sb[:], clsb_d[:])
        nc.sync.dma_start(aug_sb[:], aug_d[:])
        for u in UNITS:
            nc.vector.memset(ccar[u][:, :, :], 0.0)
            nc.vector.memset(hs[u][:, 0, :, :], 0.0)
            nc.vector.memset(hs[u][:, T + 1, :, :], 0.0)

        # mrep: broadcast host mask rows over 128 partitions via K=1 matmul
        with tc.tile_pool(name="mr", bufs=2) as mp, \
             tc.tile_pool(name="mrp", bufs=2, space="PSUM") as mps:
            ones = mp.tile([1, 128], bf16, tag="ones")
            nc.vector.memset(ones[:, :], 1.0)
            hostm_sb = {}
            for r in range(2):
                hostm_sb[r] = mp.tile([1, Bc, T], bf16, tag=f"hostm{r}", name=f"hostm{r}")
                nc.sync.dma_start(hostm_sb[r][:], hostm_d[r:r + 1, :, :])
            for r in range(2):
                for b in range(Bc):
                    psb = mps.tile([128, T], f32, tag="psb")
                    nc.tensor.matmul(out=psb[:], lhsT=ones[:, :],
                                     rhs=hostm_sb[r][:, b, :],
                                     start=True, stop=True)
                    nc.vector.tensor_copy(mrep[:, r, b, :], psb[:])

        # ---- embedding gather + transpose into xT
        with tc.tile_pool(name="gat", bufs=4) as gp, \
             tc.tile_pool(name="gps", bufs=4, space="PSUM") as gps:
            for n in range(TB // 128):
                idx = gp.tile([128, 1], i32, tag="idx")
                nc.sync.dma_start(idx[:], words_d[n * 128:(n + 1) * 128, :])
                xt = gp.tile([128, 128], f32, tag="xt")
                nc.gpsimd.indirect_dma_start(
                    out=xt[:], out_offset=None, in_=emb_d[:, :],
                    in_offset=bass.IndirectOffsetOnAxis(ap=idx[:, :1], axis=0))
                pst = gps.tile([128, 128], f32, tag="pst")
                nc.tensor.transpose(out=pst[:], in_=xt[:], identity=ident[:])
                nc.vector.tensor_copy(xT[:, n * 128:(n + 1) * 128], pst[:])

        # ---- xg precompute: xg[u] = [Wih | waug] @ [x; 1-m; 1], bf16 to DRAM
        def xg_precompute(u, rhs_of_k):
            k_cnt = KCNT[u]
            with tc.tile_pool(name=f"xp{u}", bufs=4, space="PSUM") as xps, \
                 tc.tile_pool(name=f"xs{u}", bufs=3) as xsb:
                for n in range(TB // 512):
                    nsl = slice(n * 512, (n + 1) * 512)
                    stg = xsb.tile([128, 8, 512], bf16, tag="stg")
                    for m in range(8):
                        psm = xps.tile([128, 512], f32, tag="ps")
                        for k in range(k_cnt):
                            nc.tensor.matmul(
                                out=psm[:],
                                lhsT=wx_sb[u][:, (k * 8 + m) * 128:(k * 8 + m + 1) * 128],
                                rhs=rhs_of_k(k, n),
                                start=(k == 0), stop=False)
                        nc.tensor.matmul(
                            out=psm[:],
                            lhsT=wa_sb[u][:, m * 128:(m + 1) * 128],
                            rhs=aug_sb[:, nsl],
                            start=False, stop=True)
                        if m % 2 == 0:
                            nc.vector.tensor_copy(stg[:, m, :], psm[:])
                        else:
                            nc.scalar.activation(stg[:, m, :], psm[:],
                                                 mybir.ActivationFunctionType.Copy)
                    nc.sync.dma_start(xgd[u][:, :, nsl], stg[:])

        def l1_rhs(k, n):
            return xT[:, n * 512:(n + 1) * 512]

        xg_precompute("1f", l1_rhs)
        xg_precompute("1b", l1_rhs)

        # ---- recurrence phase: two units anti-phased, fully static unroll
        def phase(units):
            with tc.tile_pool(name=f"rc{units[0]}", bufs=3) as rp, \
                 tc.tile_pool(name=f"rps{units[0]}", bufs=2, space="PSUM") as rps, \
                 tc.tile_pool(name=f"rtmp{units[0]}", bufs=3) as tp:
                xbs = {}

                def load_xb(u, body):
                    col0 = (TB - (body + 1) * SPB * Bc) if REV[u] else body * SPB * Bc
                    xbt = rp.tile([128, 8, SPB * Bc], bf16, tag=f"xb{u}", name=f"xb{u}")
                    nc.sync.dma_start(xbt[:, :, :], xgd[u][:, :, col0:col0 + SPB * Bc])
                    xbs[(u, body)] = xbt

                def step(u, s):
                    rev = REV[u]
                    body, j = s // SPB, s % SPB
                    t = (T - 1 - s) if rev else s
                    bc = ((SPB - 1 - j) if rev else j) * Bc
                    xbt = xbs[(u, body)]
                    hprev = hs[u][:, t + 2 if rev else t, :, :]
                    psm = rps.tile([128, 8, Bc], f32, tag=f"g{u}")
                    # xg into PSUM via identity matmul (clears bank)
                    nc.tensor.matmul(out=psm[:, :, :], lhsT=identb[:, :],
                                     rhs=xbt[:, :, bc:bc + Bc],
                                     start=True, stop=False)
                    for k in range(2):
                        for m in range(8):
                            nc.tensor.matmul(
                                out=psm[:, m, :],
                                lhsT=wh_sb[u][:, (k * 8 + m) * 128:(k * 8 + m + 1) * 128],
                                rhs=hprev[:, k, :],
                                start=False, stop=(k == 1 and m == 7))
                    sg = tp.tile([128, 8, Bc], bf16, tag=f"sg{u}")
                    nc.scalar.activation(sg[:, 0:6, :], psm[:, 0:6, :],
                                         mybir.ActivationFunctionType.Sigmoid)
                    nc.scalar.activation(sg[:, 6:8, :], psm[:, 6:8, :],
                                         mybir.ActivationFunctionType.Tanh)
                    t1 = tp.tile([128, 2, Bc], f32, tag=f"t1{u}")
                    nc.vector.tensor_tensor(out=t1[:, :, :], in0=sg[:, 0:2, :],
                                            in1=sg[:, 6:8, :], op=mybir.AluOpType.mult)
                    csf = tp.tile([128, 2, Bc], f32, tag=f"csf{u}")
                    nc.vector.tensor_tensor(out=csf[:, :, :], in0=sg[:, 2:4, :],
                                            in1=ccar[u][:, :, :], op=mybir.AluOpType.mult)
                    nc.vector.tensor_tensor(out=ccar[u][:, :, :], in0=csf[:, :, :],
                                            in1=t1[:, :, :], op=mybir.AluOpType.add)
                    tc2 = tp.tile([128, 2, Bc], bf16, tag=f"tc{u}")
                    nc.scalar.activation(tc2[:, :, :], ccar[u][:, :, :],
                                         mybir.ActivationFunctionType.Tanh)
                    nc.vector.tensor_tensor(out=hs[u][:, t + 1, :, :], in0=sg[:, 4:6, :],
                                            in1=tc2[:, :, :], op=mybir.AluOpType.mult)

                ua, ub = units
                for i in range(NBODY):
                    load_xb(ua, i)
                    load_xb(ub, i)
                    for j in range(SPB):
                        s = i * SPB + j
                        step(ua, s)        # leading unit at step s
                        if s >= 1:
                            step(ub, s - 1)  # trailing unit half a step behind
                step(ub, T - 1)

        phase(("1f", "1b"))

        def l2_rhs(k, n):
            src = hs["1f"] if k < 2 else hs["1b"]
            return src[:, 1 + n * 64:1 + (n + 1) * 64, k % 2, :]

        xg_precompute("2f", l2_rhs)
        xg_precompute("2b", l2_rhs)

        phase(("2f", "2b"))

        # ---- post-hoc repair of 2f outputs: o2[t] = m*h[t] + (1-m)*o2[t-1]
        with tc.tile_pool(name="rep", bufs=4) as repp:
            for b in range(Bc):
                for c in range(2):
                    mh = repp.tile([128, T], bf16, tag="mh")
                    nc.vector.tensor_tensor(out=mh[:, :], in0=hs["2f"][:, 1:T + 1, c, b],
                                            in1=mrep[:, 0, b, :], op=mybir.AluOpType.mult)
                    nc.vector.tensor_tensor_scan(
                        out=o2rep[:, :, c, b], data0=mrep[:, 1, b, :], data1=mh[:, :],
                        initial=0.0, op0=mybir.AluOpType.mult, op1=mybir.AluOpType.add)

        # ---- classifier
        with tc.tile_pool(name="cl", bufs=3) as cp, \
             tc.tile_pool(name="cps", bufs=3, space="PSUM") as cps:
            for n in range(TB // 512):
                psm = cps.tile([TAGS, 512], f32, tag="ps")
                for k in range(4):
                    if k < 2:
                        rhs = o2rep[:, n * 64:(n + 1) * 64, k % 2, :]
                    else:
                        rhs = hs["2b"][:, 1 + n * 64:1 + (n + 1) * 64, k % 2, :]
                    nc.tensor.matmul(
                        out=psm[:],
                        lhsT=cls_sb[:, k * TAGS:(k + 1) * TAGS],
                        rhs=rhs,
                        start=(k == 0), stop=(k == 3))
                lg = cp.tile([TAGS, 512], f32, tag="lg")
                nc.vector.tensor_scalar_add(lg[:], psm[:], clsb_sb[:, :1])
                nc.sync.dma_start(logits_d[:, n * 512:(n + 1) * 512], lg[:])

    nc.compile()
    return nc


def kernel(**inputs):
    words = np.asarray(inputs["words"]).astype(np.int32)      # [B, T]
    lengths = np.asarray(inputs["lengths"]).astype(np.int32)  # [B]
    emb = np.asarray(inputs["emb"], dtype=np.float32)

    if "nc" not in _CACHE:
        _CACHE["nc"] = _build_program()
    nc = _CACHE["nc"]

    mask = (lengths[:, None] > np.arange(T)[None, :]).astype(np.float32)  # [B,T]
    wprep = {u: _prep_unit_weights(inputs[f"l{u}_Wih"], inputs[f"l{u}_Whh"],
                                   inputs[f"l{u}_bih"], inputs[f"l{u}_bhh"])
             for u in UNITS}
    clsW = np.asarray(inputs["cls_W"], dtype=np.float32)      # [50, 512]
    CT = clsW.T  # [512, 50]
    clsx = np.concatenate([CT[k * 128:(k + 1) * 128, :] for k in range(4)],
                          axis=1).astype(ml_dtypes.bfloat16)  # [128, 200]
    clsb = np.asarray(inputs["cls_b"], dtype=np.float32).reshape(TAGS, 1)

    in_maps = []
    for c in range(NCORES):
        bsl = slice(c * Bc, (c + 1) * Bc)
        w_c = words[bsl]                      # [Bc, T]
        m_c = mask[bsl]                       # [Bc, T]
        words_tm = np.ascontiguousarray(w_c.T).reshape(TB, 1)
        aug = np.stack([(1.0 - m_c.T).reshape(TB), np.ones(TB, np.float32)]
                       ).astype(ml_dtypes.bfloat16)           # [2, TB]
        hostm = np.stack([m_c, 1.0 - m_c]).astype(ml_dtypes.bfloat16)  # [2, Bc, T]
        im = {"emb": emb, "words": words_tm, "aug": aug, "hostm": hostm,
              "clsx": clsx, "clsb": clsb}
        for u in UNITS:
            wx, wa, wh = wprep[u]
            im[f"w{u}x"] = wx
            im[f"w{u}a"] = wa
            im[f"w{u}h"] = wh
        in_maps.append(im)

    _CACHE["in_maps"] = in_maps
    res = run_bass_kernel_spmd(nc, in_maps, list(range(NCORES)))
    out = np.empty((B, T, TAGS), np.float32)
    for c in range(NCORES):
        lg = res.results[c]["logits"]          # [50, TB], col = t*Bc + b
        out[c * Bc:(c + 1) * Bc] = lg.reshape(TAGS, T, Bc).transpose(2, 1, 0)
    return out


def _install_ntff_hook():
    """Dev-only: register the axon NTFF profile hook that the image's antenv
    lacks, so run_bass_kernel_spmd(trace=True) can capture a profile."""
    import types
    try:
        from antenv.axon_hooks import get_axon_ntff_profile_hook
        if get_axon_ntff_profile_hook() is not None:
            return
        from antenv.axon_hooks import set_axon_ntff_profile_hook
    except ImportError:
        import antenv
        mod = types.ModuleType("antenv.axon_hooks")
        _h = [None]
        mod.set_axon_ntff_profile_hook = lambda h: _h.__setitem__(0, h)
        mod.get_axon_ntff_profile_hook = lambda: _h[0]
        antenv.axon_hooks = mod
        sys.modules["antenv.axon_hooks"] = mod
        set_axon_ntff_profile_hook = mod.set_axon_ntff_profile_hook
    if "/root/.axon_site" not in sys.path:
        sys.path.insert(0, "/root/.axon_site")
    from trn_agent_boot.trn_boot import _ntff_profile_via_ctypes
    set_axon_ntff_profile_hook(_ntff_profile_via_ctypes("/opt/axon/libaxon_pjrt.so"))


def bench(inputs):
    """Run once with NTFF tracing; returns HW exec_time_ns (and stashes trace)."""
    kernel(**inputs)  # ensure program built/cached
    nc = _CACHE["nc"]
    in_maps = _CACHE["in_maps"]
    _install_ntff_hook()
    import concourse.bass_utils as bu
    bu.upload_artifacts = lambda tmpdir: "local://" + tmpdir  # no bucket here
    import tempfile
    tmpdir = tempfile.mkdtemp(prefix="bilstm_trace_")
    res = run_bass_kernel_spmd(nc, in_maps, list(range(NCORES)), trace=True,
                               tmpdir=tmpdir)
    _CACHE["trace_dir"] = tmpdir
    _CACHE["last_bench"] = res
    print("trace dir:", tmpdir)
    return res.exec_time_ns


if __name__ == "__main__":
    import reference
    inputs = {k: np.asarray(v) for k, v in reference.setup_inputs().items()}
    got = kernel(**inputs)
    print(got.shape, got.dtype)
